# revision 1
# baseline (speedup 1.0000x reference)
"""Deductron kernel for Trainium2, 8 NeuronCores, time-sharded.

Math (matching the reference):
    h = sigmoid(W1 @ x + B1); left, right = h[:128], h[128:]
    a_t = left_t * right_t; b_t = 1 - left_t
    u_0 = 0; u_t = a_{t-1} * u_{t-1} + b_{t-1}   (z[:, t] = u_t)
    out = 1 - sigmoid(W2 @ z + B2) = sigmoid(-(W2 @ z + B2))

Sharding: the 65536-frame time axis is split into 8 chunks of 8192. Each core
also receives a 512-frame left halo. Because a_t = sigmoid(.)*sigmoid(.) < 1
and the product of 512 consecutive a's underflows to exactly 0 in fp32, the
recurrence state forgets its initial condition within the halo, so no
cross-core state exchange is needed. Core 0 has no real halo: its input is
zero-padded and a per-core input vector bscale (0 for core 0, 1 otherwise)
multiplies the halo's b values so the scan state stays exactly 0 until the
owned region starts (u_0 = 0 exactly).

The host pre-casts x to fp16 (halves input DMA; fp16's 10-bit mantissa
beats bf16 by 8x precision here since all values are O(1)) and pre-transposes the
weights into the PE's stationary layout. On-core recurrence: the Vector
engine's tensor_tensor_scan instruction computes state = a_t*state + b_t
natively along the free axis; chunk scans are chained by a 128x1 carry.
"""

import sys

for _p in ("/opt/trn_rl_repo", "/opt/pypackages"):
    if _p not in sys.path:
        sys.path.append(_p)

import numpy as np

# Problem constants (hardcoded per contract).
INPUT_LEN = 512
N_MEM = 128  # memory dim (recurrence state width) = one partition tile
OUT_LEN = 256
T_TOTAL = 65536
N_CORES = 8
T_LOC = T_TOTAL // N_CORES  # 8192 owned frames per core
HALO = 512                  # washout halo; prod(a) over 512 steps == 0 in fp32
TW = 512                    # column tile width (one PSUM bank of fp32)
W_IN = HALO + T_LOC         # per-core input width (8704)
NT = W_IN // TW             # 17 column tiles (tile 0 is pure halo)

F16_NP = np.float16


def _build_nc(t_loc=T_LOC, halo=HALO, tw=TW):
    import concourse.tile as tile
    from concourse import bacc, mybir
    from contextlib import ExitStack

    F32 = mybir.dt.float32
    F16 = mybir.dt.float16
    SIG = mybir.ActivationFunctionType.Sigmoid
    MUL = mybir.AluOpType.mult
    ADD = mybir.AluOpType.add

    w_in = halo + t_loc
    nt = w_in // tw
    assert w_in % tw == 0 and halo == tw

    nc = bacc.Bacc()
    x = nc.dram_tensor("x", [INPUT_LEN, w_in], F16, kind="ExternalInput")
    w1t = nc.dram_tensor("w1t", [N_MEM, 4 * 2 * N_MEM], F16, kind="ExternalInput")
    w2t = nc.dram_tensor("w2t", [N_MEM, OUT_LEN], F16, kind="ExternalInput")
    b1 = nc.dram_tensor("b1", [2 * N_MEM, 1], F32, kind="ExternalInput")
    negb1t = nc.dram_tensor("negb1t", [N_MEM, 1], F32, kind="ExternalInput")
    negb2 = nc.dram_tensor("negb2", [OUT_LEN, 1], F32, kind="ExternalInput")
    bscale = nc.dram_tensor("bscale", [N_MEM, 1], F32, kind="ExternalInput")
    out = nc.dram_tensor("out", [OUT_LEN, t_loc], F16, kind="ExternalOutput")

    with ExitStack() as ctx:
        tc = ctx.enter_context(tile.TileContext(nc))
        singles = ctx.enter_context(tc.tile_pool(name="singles", bufs=1))
        xpool = ctx.enter_context(tc.tile_pool(name="xpool", bufs=6))
        hpool = ctx.enter_context(tc.tile_pool(name="hpool", bufs=4))
        opool = ctx.enter_context(tc.tile_pool(name="opool", bufs=4))
        psA = ctx.enter_context(tc.tile_pool(name="psA", bufs=3, space="PSUM"))
        psB = ctx.enter_context(tc.tile_pool(name="psB", bufs=1, space="PSUM"))

        # Persistent full-width recurrence buffers. a_buf/b_buf are written at
        # a +1 column offset (a_buf[:, p] = a at input column p-1) so the scan
        # output z[:, p] = u at column p directly.
        a_buf = singles.tile([N_MEM, w_in + 1], F16)
        b_buf = singles.tile([N_MEM, w_in + 1], F16)
        z_buf = singles.tile([N_MEM, w_in], F16)

        # ---- weights / biases (host provides transposed layouts) ----
        w1t_sb = singles.tile([128, 4, 2 * N_MEM], F16)
        nc.sync.dma_start(out=w1t_sb,
                          in_=w1t[:].rearrange("p (k m) -> p k m", k=4))
        w2t_sb = singles.tile([128, 2, N_MEM], F16)
        nc.sync.dma_start(out=w2t_sb,
                          in_=w2t[:].rearrange("p (m j) -> p m j", m=2))
        b1_sb = singles.tile([128, 2, 1], F32)
        nc.sync.dma_start(out=b1_sb, in_=b1[:].rearrange("(m p) o -> p m o", p=128))
        negb1t_sb = singles.tile([128, 1], F32)
        nc.sync.dma_start(out=negb1t_sb, in_=negb1t[:])
        negb2_sb = singles.tile([128, 2, 1], F32)
        nc.sync.dma_start(out=negb2_sb,
                          in_=negb2[:].rearrange("(m p) o -> p m o", p=128))
        bs_sb = singles.tile([128, 1], F32)
        nc.sync.dma_start(out=bs_sb, in_=bscale[:])

        nc.vector.memset(a_buf[:, 0:1], 0.0)
        nc.vector.memset(b_buf[:, 0:1], 0.0)

        xr = x[:].rearrange("(k p) t -> p k t", p=128)       # (128, 4, w_in)
        outr = out[:].rearrange("(m p) t -> p m t", p=128)   # (128, 2, t_loc)

        def phase_c(j):
            # output GEMM + activation + store for z tile j (j >= 1)
            c0 = j * tw
            zr = z_buf[:, c0:c0 + tw]
            o = psB.tile([128, 2, tw], F32)
            nc.tensor.matmul(o[:, 0, :], lhsT=w2t_sb[:, 0, :], rhs=zr,
                             start=True, stop=True)
            nc.tensor.matmul(o[:, 1, :], lhsT=w2t_sb[:, 1, :], rhs=zr,
                             start=True, stop=True)
            ot = opool.tile([128, 2, tw], F16)
            nc.scalar.activation(ot[:, 0, :], o[:, 0, :], SIG,
                                 bias=negb2_sb[:, 0, :], scale=-1.0)
            nc.scalar.activation(ot[:, 1, :], o[:, 1, :], SIG,
                                 bias=negb2_sb[:, 1, :], scale=-1.0)
            nc.sync.dma_start(out=outr[:, :, c0 - halo:c0 - halo + tw], in_=ot)

        # Phase C is emitted DELAY tiles behind phases A/B: the PE stream is
        # in-order, so a GEMM2 queued right after scan j would stall the PE
        # (and everything downstream) on the serial scan spine. The delay
        # keeps the PE fed with work whose inputs are already resolved.
        DELAY = 3
        for j in range(nt):
            c0 = j * tw
            # ---- phase A: h-GEMM + activations ----
            xt = xpool.tile([128, 4, tw], F16)
            for k in range(4):
                nc.sync.dma_start(out=xt[:, k, :], in_=xr[:, k, c0:c0 + tw])
            g0 = psA.tile([128, tw], F32)
            g1 = psA.tile([128, tw], F32)
            for k in range(4):
                nc.tensor.matmul(
                    g0, lhsT=w1t_sb[:, k, 0:128], rhs=xt[:, k, :],
                    start=(k == 0), stop=(k == 3))
            for k in range(4):
                nc.tensor.matmul(
                    g1, lhsT=w1t_sb[:, k, 128:256], rhs=xt[:, k, :],
                    start=(k == 0), stop=(k == 3))
            left = hpool.tile([128, tw], F16)
            right = hpool.tile([128, tw], F16)
            nc.scalar.activation(left, g0, SIG, bias=b1_sb[:, 0, :], scale=1.0)
            nc.scalar.activation(right, g1, SIG, bias=b1_sb[:, 1, :], scale=1.0)
            # b = 1 - left on DVE (keeps the Scalar engine under the DMA roof)
            bdst = b_buf[:, c0 + 1:c0 + 1 + tw]
            nc.vector.tensor_scalar(out=bdst, in0=left,
                                    scalar1=-1.0, scalar2=1.0,
                                    op0=MUL, op1=ADD)
            nc.vector.tensor_mul(a_buf[:, c0 + 1:c0 + 1 + tw], left, right)

            if j == 0:
                # Halo b *= bscale (covers cols [0, halo]; col 0 is the memset)
                nc.vector.tensor_scalar(
                    out=b_buf[:, 0:halo + 1], in0=b_buf[:, 0:halo + 1],
                    scalar1=bs_sb[:, 0:1], scalar2=None, op0=MUL)

            # ---- phase B: recurrence scan over this tile ----
            init = 0.0 if j == 0 else z_buf[:, c0 - 1:c0]
            nc.vector.tensor_tensor_scan(
                out=z_buf[:, c0:c0 + tw],
                data0=a_buf[:, c0:c0 + tw],
                data1=b_buf[:, c0:c0 + tw],
                initial=init, op0=MUL, op1=ADD)

            # ---- phase C, delayed ----
            if j - DELAY >= 1:
                phase_c(j - DELAY)

        for j in range(max(1, nt - DELAY), nt):
            phase_c(j)

    nc.finalize()
    return nc


def _make_in_maps(inputs, W1, B1, W2, B2, t_loc=T_LOC, halo=HALO, n_cores=N_CORES):
    inputs = np.asarray(inputs, dtype=np.float32)
    W1 = np.asarray(W1, dtype=np.float32)
    B1 = np.ascontiguousarray(np.asarray(B1, dtype=np.float32))
    W2 = np.asarray(W2, dtype=np.float32)
    B2 = np.asarray(B2, dtype=np.float32)

    x_bf = inputs.astype(F16_NP)
    w1t = np.ascontiguousarray(
        W1.T.astype(F16_NP).reshape(4, 128, 2 * N_MEM)
        .transpose(1, 0, 2).reshape(128, 4 * 2 * N_MEM))      # (128, 1024)
    w2t = np.ascontiguousarray(W2.T.astype(F16_NP))          # (128, 256)
    negb1t = np.ascontiguousarray(-B1[:N_MEM])                # (128, 1)
    negb2 = np.ascontiguousarray(-B2)                         # (256, 1)

    in_maps = []
    for i in range(n_cores):
        s = i * t_loc
        lo = s - halo
        if lo < 0:
            xs = np.concatenate(
                [np.zeros((INPUT_LEN, -lo), F16_NP), x_bf[:, :s + t_loc]],
                axis=1)
        else:
            xs = x_bf[:, lo:s + t_loc]
        bs = np.full((N_MEM, 1), 0.0 if i == 0 else 1.0, np.float32)
        in_maps.append({
            "x": np.ascontiguousarray(xs),
            "w1t": w1t, "w2t": w2t, "b1": B1,
            "negb1t": negb1t, "negb2": negb2, "bscale": bs,
        })
    return in_maps


def _run(inputs, W1, B1, W2, B2, trace=False, **kw):
    from concourse.bass_utils import run_bass_kernel_spmd

    nc = _build_nc()
    in_maps = _make_in_maps(inputs, W1, B1, W2, B2)
    res = run_bass_kernel_spmd(nc, in_maps, list(range(N_CORES)), trace=trace, **kw)
    full = np.concatenate([r["out"] for r in res.results], axis=1)
    return full.astype(np.float32), res


def kernel(inputs, W1, B1, W2, B2):
    full, _ = _run(inputs, W1, B1, W2, B2, trace=False)
    return full.astype(np.float32, copy=False)



# revision 2
# speedup vs baseline: 1.0666x; 1.0666x over previous
"""Deductron kernel for Trainium2, 8 NeuronCores, time-sharded.

Math (matching the reference):
    h = sigmoid(W1 @ x + B1); left, right = h[:128], h[128:]
    a_t = left_t * right_t; b_t = 1 - left_t
    u_0 = 0; u_t = a_{t-1} * u_{t-1} + b_{t-1}   (z[:, t] = u_t)
    out = 1 - sigmoid(W2 @ z + B2) = sigmoid(-(W2 @ z + B2))

Sharding: the 65536-frame time axis is split into 8 chunks of 8192 plus a
512-frame left washout halo per core (prod of 512 consecutive a's underflows
to 0 in fp32, so no cross-core state exchange is needed; core 0's halo input
is zero-padded and its halo b is scaled by 0 so the state stays exactly 0).

v2 changes vs the fp16 baseline:
  * GEMM1 runs in fp8e4 (e4m3) with MatmulPerfMode.DoubleRow: 256-deep
    contraction per matmul at 0.5 PE cycles/row -> 4x less PE time. W1 is
    pre-scaled by 8 on host (avoids fp8 subnormals); the h-activation applies
    scale=1/8. Host-emulated end-to-end rel err: 2.0e-3 (gate 2e-2).
  * Frames processed in PAIRS of 512-col tiles: one contiguous 512KB input
    DMA per pair (128 descriptors x 4KB -> full DMA-engine rate, and 9+8+3
    total DMA instructions instead of 90 -- DMA issue costs 565ns each of
    serial Sync-engine time).
  * Activations batched 1024 wide over 4-bank PSUM tiles (ACT instruction
    overhead ~250ns each; the Activation engine is the end bottleneck).
  * b = 1-left computed on GpSimd; a = left*right and the recurrence scan on
    DVE; output sigmoids+h sigmoids on Scalar -- balances the four engines.
"""

import sys

for _p in ("/opt/trn_rl_repo", "/opt/pypackages"):
    if _p not in sys.path:
        sys.path.append(_p)

import numpy as np
import ml_dtypes

# Problem constants (hardcoded per contract).
INPUT_LEN = 512
N_MEM = 128
OUT_LEN = 256
T_TOTAL = 65536
N_CORES = 8
T_LOC = T_TOTAL // N_CORES   # 8192 owned frames per core
HALO = 512                   # washout halo; prod(a) over 512 steps == 0 in fp32
TW = 512                     # column tile width (one PSUM bank of fp32)
NPAIR = T_LOC // (2 * TW)    # 8 pairs of owned tiles per core
W_IN = HALO + T_LOC          # 8704
W1_SCALE = 8.0               # host multiplies W1 by this; ACT applies 1/8

F16_NP = np.float16
F8_NP = ml_dtypes.float8_e4m3fn


def _build_nc():
    import concourse.tile as tile
    from concourse import bacc, mybir
    from contextlib import ExitStack

    F32 = mybir.dt.float32
    F16 = mybir.dt.float16
    F8 = mybir.dt.float8e4
    SIG = mybir.ActivationFunctionType.Sigmoid
    MUL = mybir.AluOpType.mult
    ADD = mybir.AluOpType.add
    DR = mybir.MatmulPerfMode.DoubleRow

    nc = bacc.Bacc()
    # DRAM layouts are host-packed so every DMA is fully contiguous.
    x_halo = nc.dram_tensor("x_halo", [128, 2048], F8, kind="ExternalInput")
    x_main = nc.dram_tensor("x_main", [NPAIR, 128, 4096], F8, kind="ExternalInput")
    w1 = nc.dram_tensor("w1", [128, 1024], F8, kind="ExternalInput")
    w2 = nc.dram_tensor("w2", [128, 256], F16, kind="ExternalInput")
    bias = nc.dram_tensor("bias", [128, 5], F32, kind="ExternalInput")
    out = nc.dram_tensor("out", [NPAIR, 128, 2048], F16, kind="ExternalOutput")

    with ExitStack() as ctx:
        tc = ctx.enter_context(tile.TileContext(nc))
        singles = ctx.enter_context(tc.tile_pool(name="singles", bufs=1))
        xpool = ctx.enter_context(tc.tile_pool(name="xpool", bufs=3))
        hpool = ctx.enter_context(tc.tile_pool(name="hpool", bufs=3))
        opool = ctx.enter_context(tc.tile_pool(name="opool", bufs=2))
        psG = ctx.enter_context(tc.tile_pool(name="psG", bufs=1, space="PSUM"))
        psO = ctx.enter_context(tc.tile_pool(name="psO", bufs=1, space="PSUM"))

        # Persistent recurrence buffers. a_buf/b_buf are written at a +1
        # column offset (a_buf[:, p] = a at input column p-1) so the scan
        # output z[:, p] = u at column p directly.
        a_buf = singles.tile([N_MEM, W_IN + 1], F16)
        b_buf = singles.tile([N_MEM, W_IN + 1], F16)
        z_buf = singles.tile([N_MEM, W_IN], F16)

        # ---- weights / biases (host provides packed layouts) ----
        # w1_sb[p, c, i, h, m] = 8*W1[h*128+m, c*256 + i*128 + p]
        w1_sb = singles.tile([128, 2, 2, 2, 128], F8)
        nc.sync.dma_start(out=w1_sb,
                          in_=w1[:].rearrange("p (c i h m) -> p c i h m",
                                              c=2, i=2, h=2))
        # w2_sb[p, h, m] = W2[h*128+m, p]
        w2_sb = singles.tile([128, 2, 128], F16)
        nc.sync.dma_start(out=w2_sb,
                          in_=w2[:].rearrange("p (h m) -> p h m", h=2))
        # bias cols: [B1a, B1b, -B2a, -B2b, bscale]
        bias_sb = singles.tile([128, 5], F32)
        nc.sync.dma_start(out=bias_sb, in_=bias[:])

        nc.vector.memset(a_buf[:, 0:1], 0.0)
        nc.vector.memset(b_buf[:, 0:1], 0.0)

        xmr = x_main[:].rearrange("q p (t c i w) -> q p t c i w", t=2, c=2, i=2)
        outr = out[:].rearrange("q p (h t w) -> q p h t w", h=2, t=2)

        def phase_c(q):
            # output GEMM + activation + store for pair q (z cols
            # [512+1024q, 512+1024q+1024), out cols [1024q, 1024q+1024))
            zc = HALO + 1024 * q
            o = psO.tile([128, 2, 2, TW], F32)
            for h in range(2):
                for t in range(2):
                    nc.tensor.matmul(o[:, h, t, :], lhsT=w2_sb[:, h, :],
                                     rhs=z_buf[:, zc + TW * t:zc + TW * (t + 1)],
                                     start=True, stop=True)
            ot = opool.tile([128, 2, 2, TW], F16)
            nc.scalar.activation(ot[:, 0, :, :], o[:, 0, :, :], SIG,
                                 bias=bias_sb[:, 2:3], scale=-1.0)
            nc.scalar.activation(ot[:, 1, :, :], o[:, 1, :, :], SIG,
                                 bias=bias_sb[:, 3:4], scale=-1.0)
            nc.sync.dma_start(out=outr[q], in_=ot)

        DELAY = 2  # pairs of lead distance between phase A/B and phase C

        # ---- halo tile (columns [0, 512)) ----
        xt = xpool.tile([128, 2, 2, 2, TW], F8)
        nc.sync.dma_start(out=xt[:, 0, :, :, :],
                          in_=x_halo[:].rearrange("p (c i w) -> p c i w",
                                                  c=2, i=2))
        g = psG.tile([128, 2, 2, TW], F32)
        for c in range(2):
            for h in range(2):
                nc.tensor.matmul(g[:, h, 0, :], lhsT=w1_sb[:, c, :, h, :],
                                 rhs=xt[:, 0, c, :, :],
                                 start=(c == 0), stop=(c == 1), perf_mode=DR)
        left = hpool.tile([128, 2, TW], F16)
        right = hpool.tile([128, 2, TW], F16)
        nc.scalar.activation(left[:, 0, :], g[:, 0, 0, :], SIG,
                             bias=bias_sb[:, 0:1], scale=1.0 / W1_SCALE)
        nc.scalar.activation(right[:, 0, :], g[:, 1, 0, :], SIG,
                             bias=bias_sb[:, 1:2], scale=1.0 / W1_SCALE)
        nc.gpsimd.tensor_scalar(out=b_buf[:, 1:1 + TW], in0=left[:, 0, :],
                                scalar1=-1.0, scalar2=1.0, op0=MUL, op1=ADD)
        nc.vector.tensor_tensor(out=a_buf[:, 1:1 + TW], in0=left[:, 0, :],
                                in1=right[:, 0, :], op=MUL)
        # Halo b *= bscale (0 on core 0 so the state stays exactly 0)
        nc.vector.tensor_scalar(out=b_buf[:, 0:HALO + 1],
                                in0=b_buf[:, 0:HALO + 1],
                                scalar1=bias_sb[:, 4:5], scalar2=None, op0=MUL)
        nc.vector.tensor_tensor_scan(out=z_buf[:, 0:HALO],
                                     data0=a_buf[:, 0:HALO],
                                     data1=b_buf[:, 0:HALO],
                                     initial=0.0, op0=MUL, op1=ADD)

        # ---- owned pairs ----
        for p in range(NPAIR):
            c0 = HALO + 1024 * p
            xt = xpool.tile([128, 2, 2, 2, TW], F8)
            nc.sync.dma_start(out=xt, in_=xmr[p])
            g = psG.tile([128, 2, 2, TW], F32)
            for c in range(2):
                for h in range(2):
                    for t in range(2):
                        nc.tensor.matmul(
                            g[:, h, t, :], lhsT=w1_sb[:, c, :, h, :],
                            rhs=xt[:, t, c, :, :],
                            start=(c == 0), stop=(c == 1), perf_mode=DR)
            left = hpool.tile([128, 2, TW], F16)
            right = hpool.tile([128, 2, TW], F16)
            nc.scalar.activation(left, g[:, 0, :, :], SIG,
                                 bias=bias_sb[:, 0:1], scale=1.0 / W1_SCALE)
            nc.scalar.activation(right, g[:, 1, :, :], SIG,
                                 bias=bias_sb[:, 1:2], scale=1.0 / W1_SCALE)
            lf = left[:].rearrange("p a b -> p (a b)")
            rf = right[:].rearrange("p a b -> p (a b)")
            nc.gpsimd.tensor_scalar(out=b_buf[:, c0 + 1:c0 + 1025], in0=lf,
                                    scalar1=-1.0, scalar2=1.0,
                                    op0=MUL, op1=ADD)
            nc.vector.tensor_tensor(out=a_buf[:, c0 + 1:c0 + 1025],
                                    in0=lf, in1=rf, op=MUL)
            nc.vector.tensor_tensor_scan(out=z_buf[:, c0:c0 + 1024],
                                         data0=a_buf[:, c0:c0 + 1024],
                                         data1=b_buf[:, c0:c0 + 1024],
                                         initial=z_buf[:, c0 - 1:c0],
                                         op0=MUL, op1=ADD)
            if p - DELAY >= 0:
                phase_c(p - DELAY)

        for q in range(max(0, NPAIR - DELAY), NPAIR):
            phase_c(q)

    nc.finalize()
    return nc


def _make_in_maps(inputs, W1, B1, W2, B2):
    inputs = np.asarray(inputs, dtype=np.float32)
    W1 = np.asarray(W1, dtype=np.float32)
    B1 = np.asarray(B1, dtype=np.float32)
    W2 = np.asarray(W2, dtype=np.float32)
    B2 = np.asarray(B2, dtype=np.float32)

    x8 = inputs.astype(F8_NP)
    # w1[p, c, i, h, m] = 8*W1[h*128+m, c*256+i*128+p]
    w1p = np.ascontiguousarray(
        (W1 * W1_SCALE).astype(F8_NP)
        .reshape(2, 128, 2, 2, 128)            # h, m, c, i, p
        .transpose(4, 2, 3, 0, 1)              # p, c, i, h, m
        .reshape(128, 1024))
    # w2[p, h, m] = W2[h*128+m, p]
    w2p = np.ascontiguousarray(
        W2.astype(F16_NP).reshape(2, 128, 128)  # h, m, p
        .transpose(2, 0, 1).reshape(128, 256))
    biasc = np.empty((128, 5), np.float32)
    biasc[:, 0] = B1[:128, 0]
    biasc[:, 1] = B1[128:, 0]
    biasc[:, 2] = -B2[:128, 0]
    biasc[:, 3] = -B2[128:, 0]

    in_maps = []
    for i in range(N_CORES):
        s = i * T_LOC
        lo = s - HALO
        if lo < 0:
            xs = np.concatenate(
                [np.zeros((INPUT_LEN, -lo), F8_NP), x8[:, :s + T_LOC]], axis=1)
        else:
            xs = x8[:, lo:s + T_LOC]
        xr = xs.reshape(2, 2, 128, W_IN)                  # c, i, p, col
        xh = np.ascontiguousarray(
            xr[:, :, :, :HALO].transpose(2, 0, 1, 3).reshape(128, 2048))
        xm = np.ascontiguousarray(
            xr[:, :, :, HALO:].reshape(2, 2, 128, NPAIR, 2, TW)
            .transpose(3, 2, 4, 0, 1, 5)                  # pair, p, t, c, i, w
            .reshape(NPAIR, 128, 4096))
        b = biasc.copy()
        b[:, 4] = 0.0 if i == 0 else 1.0
        in_maps.append({
            "x_halo": xh, "x_main": xm,
            "w1": w1p, "w2": w2p, "bias": b,
        })
    return in_maps


def _run(inputs, W1, B1, W2, B2, trace=False, **kw):
    from concourse.bass_utils import run_bass_kernel_spmd

    nc = _build_nc()
    in_maps = _make_in_maps(inputs, W1, B1, W2, B2)
    res = run_bass_kernel_spmd(nc, in_maps, list(range(N_CORES)), trace=trace, **kw)
    parts = []
    for r in res.results:
        o = np.asarray(r["out"]).astype(np.float32)       # (NPAIR, 128, 2048)
        o = (o.reshape(NPAIR, 128, 2, 2, TW)              # pair, p, h, t, w
             .transpose(2, 1, 0, 3, 4)                    # h, p, pair, t, w
             .reshape(OUT_LEN, T_LOC))
        parts.append(o)
    full = np.concatenate(parts, axis=1)
    return full, res


def kernel(inputs, W1, B1, W2, B2):
    full, _ = _run(inputs, W1, B1, W2, B2, trace=False)
    return full.astype(np.float32, copy=False)


# revision 11
# speedup vs baseline: 1.2446x; 1.1669x over previous
"""Deductron kernel for Trainium2, 8 NeuronCores, time-sharded.

Math (matching the reference):
    h = sigmoid(W1 @ x + B1); left, right = h[:128], h[128:]
    a_t = left_t * right_t; b_t = 1 - left_t
    u_0 = 0; u_t = a_{t-1} * u_{t-1} + b_{t-1}   (z[:, t] = u_t)
    out = 1 - sigmoid(W2 @ z + B2) = sigmoid(-(W2 @ z + B2))

Sharding: the 65536-frame time axis is split into 8 chunks of 8192 plus a
512-frame left washout halo per core (prod of 512 consecutive a's underflows
to 0 in fp32, so no cross-core state exchange is needed; core 0's halo input
is zero-padded and its halo b is scaled by 0 so the state stays exactly 0).

v2 changes vs the fp16 baseline:
  * GEMM1 runs in fp8e4 (e4m3) with MatmulPerfMode.DoubleRow: 256-deep
    contraction per matmul at 0.5 PE cycles/row -> 4x less PE time. W1 is
    pre-scaled by 8 on host (avoids fp8 subnormals); the h-activation applies
    scale=1/8. Host-emulated end-to-end rel err: 2.0e-3 (gate 2e-2).
  * Frames processed in PAIRS of 512-col tiles: one contiguous 512KB input
    DMA per pair (128 descriptors x 4KB -> full DMA-engine rate, and 9+8+3
    total DMA instructions instead of 90 -- DMA issue costs 565ns each of
    serial Sync-engine time).
  * Activations batched 1024 wide over 4-bank PSUM tiles (ACT instruction
    overhead ~250ns each; the Activation engine is the end bottleneck).
  * b = 1-left computed on GpSimd; a = left*right and the recurrence scan on
    DVE; output sigmoids+h sigmoids on Scalar -- balances the four engines.
"""

import sys

for _p in ("/opt/trn_rl_repo", "/opt/pypackages"):
    if _p not in sys.path:
        sys.path.append(_p)

import numpy as np
import ml_dtypes

# Problem constants (hardcoded per contract).
INPUT_LEN = 512
N_MEM = 128
OUT_LEN = 256
T_TOTAL = 65536
N_CORES = 8
T_LOC = T_TOTAL // N_CORES   # 8192 owned frames per core
HALO = 512                   # washout halo; prod(a) over 512 steps == 0 in fp32
TW = 512                     # column tile width (one PSUM bank of fp32)
NPAIR = T_LOC // (2 * TW)    # 8 pairs of owned tiles per core
W_IN = HALO + T_LOC          # 8704
W1_SCALE = 8.0               # host multiplies W1 by this; ACT applies 1/8

F16_NP = np.float16
F8_NP = ml_dtypes.float8_e4m3fn


def _build_nc():
    import concourse.tile as tile
    from concourse import bacc, mybir
    from contextlib import ExitStack

    F32 = mybir.dt.float32
    F16 = mybir.dt.float16
    F8 = mybir.dt.float8e4
    SIG = mybir.ActivationFunctionType.Sigmoid
    MUL = mybir.AluOpType.mult
    ADD = mybir.AluOpType.add
    DR = mybir.MatmulPerfMode.DoubleRow

    nc = bacc.Bacc()
    # DRAM layouts are host-packed so every DMA is fully contiguous.
    x_halo = nc.dram_tensor("x_halo", [128, 2048], F8, kind="ExternalInput")
    x_main = nc.dram_tensor("x_main", [NPAIR, 128, 4096], F8, kind="ExternalInput")
    w1 = nc.dram_tensor("w1", [128, 1024], F8, kind="ExternalInput")
    w2 = nc.dram_tensor("w2", [128, 256], F16, kind="ExternalInput")
    bias = nc.dram_tensor("bias", [128, 5], F32, kind="ExternalInput")
    out = nc.dram_tensor("out", [NPAIR, 128, 2048], F16, kind="ExternalOutput")

    with ExitStack() as ctx:
        tc = ctx.enter_context(tile.TileContext(nc))
        singles = ctx.enter_context(tc.tile_pool(name="singles", bufs=1))
        xpool = ctx.enter_context(tc.tile_pool(name="xpool", bufs=3))
        hpool = ctx.enter_context(tc.tile_pool(name="hpool", bufs=3))
        opool = ctx.enter_context(tc.tile_pool(name="opool", bufs=2))
        psG = ctx.enter_context(tc.tile_pool(name="psG", bufs=2, space="PSUM"))
        psO = ctx.enter_context(tc.tile_pool(name="psO", bufs=2, space="PSUM"))

        # Persistent recurrence buffers. a_buf/b_buf are written at a +1
        # column offset (a_buf[:, p] = a at input column p-1) so the scan
        # output z[:, p] = u at column p directly.
        a_buf = singles.tile([N_MEM, W_IN + 1], F16)
        b_buf = singles.tile([N_MEM, W_IN + 1], F16)
        z_buf = singles.tile([N_MEM, W_IN], F16)

        # ---- weights / biases (host provides packed layouts) ----
        # w1_sb[p, c, i, h, m] = 8*W1[h*128+m, c*256 + i*128 + p]
        w1_sb = singles.tile([128, 2, 2, 2, 128], F8)
        nc.sync.dma_start(out=w1_sb,
                          in_=w1[:].rearrange("p (c i h m) -> p c i h m",
                                              c=2, i=2, h=2))
        # w2_sb[p, h, m] = W2[h*128+m, p]
        w2_sb = singles.tile([128, 2, 128], F16)
        nc.sync.dma_start(out=w2_sb,
                          in_=w2[:].rearrange("p (h m) -> p h m", h=2))
        # bias cols: [B1a, B1b, -B2a, -B2b, bscale]
        bias_sb = singles.tile([128, 5], F32)
        nc.sync.dma_start(out=bias_sb, in_=bias[:])

        nc.vector.memset(a_buf[:, 0:1], 0.0)
        nc.vector.memset(b_buf[:, 0:1], 0.0)

        xmr = x_main[:].rearrange("q p (t c i w) -> q p t c i w", t=2, c=2, i=2)
        outr = out[:].rearrange("q p (h t w) -> q p h t w", h=2, t=2)

        def phase_c(q):
            # output GEMM + activation + store for pair q (z cols
            # [512+1024q, 512+1024q+1024), out cols [1024q, 1024q+1024))
            zc = HALO + 1024 * q
            ot = opool.tile([128, 2, 2, TW], F16)
            for t in range(2):
                o = psO.tile([128, 2, TW], F32)
                for h in range(2):
                    nc.tensor.matmul(o[:, h, :], lhsT=w2_sb[:, h, :],
                                     rhs=z_buf[:, zc + TW * t:zc + TW * (t + 1)],
                                     start=True, stop=True)
                nc.scalar.activation(ot[:, 0, t, :], o[:, 0, :], SIG,
                                     bias=bias_sb[:, 2:3], scale=-1.0)
                nc.scalar.activation(ot[:, 1, t, :], o[:, 1, :], SIG,
                                     bias=bias_sb[:, 3:4], scale=-1.0)
            nc.sync.dma_start(out=outr[q], in_=ot)

        DELAY = 2  # pairs of lead distance between phase A/B and phase C

        # ---- halo tile (columns [0, 512)) ----
        xt = xpool.tile([128, 2, 2, 2, TW], F8)
        nc.sync.dma_start(out=xt[:, 0, :, :, :],
                          in_=x_halo[:].rearrange("p (c i w) -> p c i w",
                                                  c=2, i=2))
        g = psG.tile([128, 2, TW], F32)
        for c in range(2):
            for h in range(2):
                nc.tensor.matmul(g[:, h, :], lhsT=w1_sb[:, c, :, h, :],
                                 rhs=xt[:, 0, c, :, :],
                                 start=(c == 0), stop=(c == 1), perf_mode=DR)
        left = hpool.tile([128, 2, TW], F16)
        right = hpool.tile([128, 2, TW], F16)
        nc.scalar.activation(left[:, 0, :], g[:, 0, :], SIG,
                             bias=bias_sb[:, 0:1], scale=1.0 / W1_SCALE)
        nc.scalar.activation(right[:, 0, :], g[:, 1, :], SIG,
                             bias=bias_sb[:, 1:2], scale=1.0 / W1_SCALE)
        nc.gpsimd.tensor_scalar(out=b_buf[:, 1:1 + TW], in0=left[:, 0, :],
                                scalar1=-1.0, scalar2=1.0, op0=MUL, op1=ADD)
        nc.vector.tensor_tensor(out=a_buf[:, 1:1 + TW], in0=left[:, 0, :],
                                in1=right[:, 0, :], op=MUL)
        # Halo b *= bscale (0 on core 0 so the state stays exactly 0)
        nc.vector.tensor_scalar(out=b_buf[:, 0:HALO + 1],
                                in0=b_buf[:, 0:HALO + 1],
                                scalar1=bias_sb[:, 4:5], scalar2=None, op0=MUL)
        nc.vector.tensor_tensor_scan(out=z_buf[:, 0:HALO],
                                     data0=a_buf[:, 0:HALO],
                                     data1=b_buf[:, 0:HALO],
                                     initial=0.0, op0=MUL, op1=ADD)

        # ---- owned pairs ----
        for p in range(NPAIR):
            c0 = HALO + 1024 * p
            xt = xpool.tile([128, 2, 2, 2, TW], F8)
            nc.sync.dma_start(out=xt, in_=xmr[p])
            left = hpool.tile([128, 2, TW], F16)
            right = hpool.tile([128, 2, TW], F16)
            for t in range(2):
                g = psG.tile([128, 2, TW], F32)
                for c in range(2):
                    for h in range(2):
                        nc.tensor.matmul(
                            g[:, h, :], lhsT=w1_sb[:, c, :, h, :],
                            rhs=xt[:, t, c, :, :],
                            start=(c == 0), stop=(c == 1), perf_mode=DR)
                nc.scalar.activation(left[:, t, :], g[:, 0, :], SIG,
                                     bias=bias_sb[:, 0:1], scale=1.0 / W1_SCALE)
                nc.scalar.activation(right[:, t, :], g[:, 1, :], SIG,
                                     bias=bias_sb[:, 1:2], scale=1.0 / W1_SCALE)
            lf = left[:].rearrange("p a b -> p (a b)")
            rf = right[:].rearrange("p a b -> p (a b)")
            nc.gpsimd.tensor_scalar(out=b_buf[:, c0 + 1:c0 + 1025], in0=lf,
                                    scalar1=-1.0, scalar2=1.0,
                                    op0=MUL, op1=ADD)
            nc.vector.tensor_tensor(out=a_buf[:, c0 + 1:c0 + 1025],
                                    in0=lf, in1=rf, op=MUL)
            nc.vector.tensor_tensor_scan(out=z_buf[:, c0:c0 + 1024],
                                         data0=a_buf[:, c0:c0 + 1024],
                                         data1=b_buf[:, c0:c0 + 1024],
                                         initial=z_buf[:, c0 - 1:c0],
                                         op0=MUL, op1=ADD)
            if p - DELAY >= 0:
                phase_c(p - DELAY)

        for q in range(max(0, NPAIR - DELAY), NPAIR):
            phase_c(q)

    nc.finalize()
    return nc


def _make_in_maps(inputs, W1, B1, W2, B2):
    inputs = np.asarray(inputs, dtype=np.float32)
    W1 = np.asarray(W1, dtype=np.float32)
    B1 = np.asarray(B1, dtype=np.float32)
    W2 = np.asarray(W2, dtype=np.float32)
    B2 = np.asarray(B2, dtype=np.float32)

    x8 = inputs.astype(F8_NP)
    # w1[p, c, i, h, m] = 8*W1[h*128+m, c*256+i*128+p]
    w1p = np.ascontiguousarray(
        (W1 * W1_SCALE).astype(F8_NP)
        .reshape(2, 128, 2, 2, 128)            # h, m, c, i, p
        .transpose(4, 2, 3, 0, 1)              # p, c, i, h, m
        .reshape(128, 1024))
    # w2[p, h, m] = W2[h*128+m, p]
    w2p = np.ascontiguousarray(
        W2.astype(F16_NP).reshape(2, 128, 128)  # h, m, p
        .transpose(2, 0, 1).reshape(128, 256))
    biasc = np.empty((128, 5), np.float32)
    biasc[:, 0] = B1[:128, 0]
    biasc[:, 1] = B1[128:, 0]
    biasc[:, 2] = -B2[:128, 0]
    biasc[:, 3] = -B2[128:, 0]

    in_maps = []
    for i in range(N_CORES):
        s = i * T_LOC
        lo = s - HALO
        if lo < 0:
            xs = np.concatenate(
                [np.zeros((INPUT_LEN, -lo), F8_NP), x8[:, :s + T_LOC]], axis=1)
        else:
            xs = x8[:, lo:s + T_LOC]
        xr = xs.reshape(2, 2, 128, W_IN)                  # c, i, p, col
        xh = np.ascontiguousarray(
            xr[:, :, :, :HALO].transpose(2, 0, 1, 3).reshape(128, 2048))
        xm = np.ascontiguousarray(
            xr[:, :, :, HALO:].reshape(2, 2, 128, NPAIR, 2, TW)
            .transpose(3, 2, 4, 0, 1, 5)                  # pair, p, t, c, i, w
            .reshape(NPAIR, 128, 4096))
        b = biasc.copy()
        b[:, 4] = 0.0 if i == 0 else 1.0
        in_maps.append({
            "x_halo": xh, "x_main": xm,
            "w1": w1p, "w2": w2p, "bias": b,
        })
    return in_maps


def _run(inputs, W1, B1, W2, B2, trace=False, **kw):
    from concourse.bass_utils import run_bass_kernel_spmd

    nc = _build_nc()
    in_maps = _make_in_maps(inputs, W1, B1, W2, B2)
    res = run_bass_kernel_spmd(nc, in_maps, list(range(N_CORES)), trace=trace, **kw)
    parts = []
    for r in res.results:
        o = np.asarray(r["out"]).astype(np.float32)       # (NPAIR, 128, 2048)
        o = (o.reshape(NPAIR, 128, 2, 2, TW)              # pair, p, h, t, w
             .transpose(2, 1, 0, 3, 4)                    # h, p, pair, t, w
             .reshape(OUT_LEN, T_LOC))
        parts.append(o)
    full = np.concatenate(parts, axis=1)
    return full, res


def kernel(inputs, W1, B1, W2, B2):
    full, _ = _run(inputs, W1, B1, W2, B2, trace=False)
    return full.astype(np.float32, copy=False)
